# revision 10
# baseline (speedup 1.0000x reference)
# Trainium2 Bass kernel for nn_AttentionModuleAggregator_48490180772189.
#
# Math: the reference applies softmax over a size-1 axis, so the attention
# map is exactly all-ones and the weighted sum collapses to
#   result[k, m] = sum_n x[k, n, m]          (x: [K=32, N=8192, M=256] f32)
#   attention_map = ones([K, N])
# The gated-MLP scores are dead code for the output, so the kernel is a pure
# memory-bound reduction over N: read 256 MiB of x once.
#
# Sharding: data-parallel over the bag axis K — 4 bags per core on 8 cores
# (32 MiB per core, contiguous slices, no host-side reduction).
#
# Per-core kernel: x shard viewed as [16, 128, 4096] (2 MiB SBUF tiles, each
# a contiguous DRAM block). TensorE reduces across partitions by matmul with
# a ones[128,1] stationary vector (float32r moving operand: 1 cycle/row),
# accumulating each bag's 32 matmuls into one PSUM [1,512] bank. A DVE fold
# adds the two 256-wide halves (even/odd n rows), and one 4 KiB DMA writes
# the [4, 256] per-core result.

import numpy as np

K, N, M = 32, 8192, 256
NCORES = 8
KPC = K // NCORES  # bags per core
P = 128  # SBUF partitions
FREE = 2048  # free-dim per x tile (1 MiB tiles)
TILE_ELEMS = P * FREE
NT = (KPC * N * M) // TILE_ELEMS  # 16 tiles per core
TILES_PER_BAG = (N * M) // TILE_ELEMS  # 4
MM_FREE = 512  # matmul moving free dim (one PSUM bank)
SLICES = FREE // MM_FREE  # 8 matmuls per tile

_cache = {}


def _build_nc(reps=1):
    import contextlib

    import concourse.bass as bass
    import concourse.tile as tile
    from concourse import bacc, mybir

    F32 = mybir.dt.float32
    F32R = mybir.dt.float32r

    nc = bacc.Bacc("TRN2", target_bir_lowering=False, debug=False)
    # float32r end-to-end: same bytes as float32 (numpy side sees float32),
    # but lets the PE run the single-pass fp32 matmul mode (1 cycle/row
    # instead of 4) and keeps the BIR verifier's fp32r producer-chain check
    # happy (DMA → SBUF tile → matmul all typed fp32r).
    x_d = nc.dram_tensor("x", [NT, P, FREE], F32R, kind="ExternalInput")
    ones_d = nc.dram_tensor("ones", [P, 1], F32R, kind="ExternalInput")
    out_d = nc.dram_tensor("out", [1, KPC * M], F32, kind="ExternalOutput")

    with tile.TileContext(nc) as tc:
        with (
            tc.tile_pool(name="xin", bufs=6) as xpool,
            tc.tile_pool(name="const", bufs=1) as cpool,
            tc.tile_pool(name="res", bufs=1) as rpool,
            tc.tile_pool(name="accp", bufs=KPC, space=bass.MemorySpace.PSUM) as ppool,
        ):
            ones = cpool.tile([P, 1], F32R)
            nc.sync.dma_start(out=ones[:], in_=ones_d[:])
            res = rpool.tile([1, KPC * M], F32)

            # reps>1 wraps the body in a hardware loop — used only by the
            # benchmark harness to amplify kernel time over the ~80 ms axon
            # RPC floor; the graded kernel uses reps=1 (no loop).
            loop = tc.For_i(0, reps, 1) if reps > 1 else contextlib.nullcontext()
            with loop:
                for b in range(KPC):
                    acc = ppool.tile([1, MM_FREE], F32)
                    nmm = TILES_PER_BAG * SLICES
                    idx = 0
                    for t in range(TILES_PER_BAG):
                        xt = xpool.tile([P, FREE], F32R)
                        nc.sync.dma_start(out=xt[:], in_=x_d[b * TILES_PER_BAG + t])
                        for j in range(SLICES):
                            nc.tensor.matmul(
                                acc[0:1, :],
                                ones[:, 0:1],
                                xt[:, j * MM_FREE : (j + 1) * MM_FREE],
                                start=(idx == 0),
                                stop=(idx == nmm - 1),
                            )
                            idx += 1
                    # psum[0, c] holds sums for (n-parity c//256, m = c%256).
                    rb = res[0:1, b * M : (b + 1) * M]
                    nc.vector.tensor_copy(rb, acc[0:1, 0:M])
                    nc.vector.tensor_add(rb, rb, acc[0:1, M : 2 * M])

                nc.sync.dma_start(out=out_d[:], in_=res[:])

    nc.compile()
    return nc


def _get_nc():
    if "nc" not in _cache:
        _cache["nc"] = _build_nc()
    return _cache["nc"]


def _run_on_cores(x, trace=False):
    from concourse.bass_utils import run_bass_kernel_spmd

    nc = _get_nc()
    x = np.ascontiguousarray(np.asarray(x, dtype=np.float32))
    assert x.shape == (K, N, M)
    ones_in = np.ones((P, 1), dtype=np.float32)
    in_maps = [
        {"x": x[c * KPC : (c + 1) * KPC].reshape(NT, P, FREE), "ones": ones_in}
        for c in range(NCORES)
    ]
    r = run_bass_kernel_spmd(
        nc, in_maps, core_ids=list(range(NCORES)), trace=trace
    )
    result = np.concatenate(
        [r.results[c]["out"].reshape(KPC, M) for c in range(NCORES)], axis=0
    )
    return result, r


def kernel(x, W_left=None, b_left=None, W_right=None, b_right=None, w_last=None,
           b_last=None, **_unused):
    result, _ = _run_on_cores(x, trace=False)
    attention_map = np.ones((K, N), dtype=np.float32)
    return result, attention_map


# revision 11
# speedup vs baseline: 2.4154x; 2.4154x over previous
# Trainium2 Bass kernel for nn_AttentionModuleAggregator_48490180772189.
#
# Math: the reference applies softmax over a size-1 axis, so the attention
# map is exactly all-ones and the weighted sum collapses to
#   result[k, m] = sum_n x[k, n, m]          (x: [K=32, N=8192, M=256] f32)
#   attention_map = ones([K, N])
# The gated-MLP scores are dead code for the output, so the kernel is a pure
# memory-bound reduction over N: read 256 MiB of x once.
#
# Sharding: data-parallel over the bag axis K — 4 bags per core on 8 cores
# (32 MiB per core, contiguous slices, no host-side reduction).
#
# Per-core kernel: x shard viewed as [32, 128, 2048] (1 MiB SBUF tiles, each
# a contiguous DRAM block; 1 MiB transfers measured fastest — 256 KiB pays
# descriptor overhead, >=2 MiB loses overlap). TensorE reduces across
# partitions by matmul with a ones[128,1] stationary vector (float32r moving
# operand: 1 cycle/row vs 4 for fp32), accumulating each bag's 32 matmuls
# into one PSUM [1,512] bank. A DVE fold adds the two 256-wide halves
# (even/odd n-row parity), and one 4 KiB DMA writes the [4, 256] result.
#
# Measured on trn2 (in-NEFF loop, slope between rep counts): 101.5 us/iter,
# vs 100.8 us for the bare DMA stream with no compute — i.e. within ~1% of
# this hardware's achievable HBM read floor (32 MiB/core, 8 cores busy).

import numpy as np

K, N, M = 32, 8192, 256
NCORES = 8
KPC = K // NCORES  # bags per core
P = 128  # SBUF partitions
FREE = 2048  # free-dim per x tile (1 MiB tiles)
TILE_ELEMS = P * FREE
NT = (KPC * N * M) // TILE_ELEMS  # 16 tiles per core
TILES_PER_BAG = (N * M) // TILE_ELEMS  # 4
MM_FREE = 512  # matmul moving free dim (one PSUM bank)
SLICES = FREE // MM_FREE  # 8 matmuls per tile

_cache = {}


def _build_nc(reps=1):
    import contextlib

    import concourse.bass as bass
    import concourse.tile as tile
    from concourse import bacc, mybir

    F32 = mybir.dt.float32
    F32R = mybir.dt.float32r

    nc = bacc.Bacc("TRN2", target_bir_lowering=False, debug=False)
    # float32r end-to-end: same bytes as float32 (numpy side sees float32),
    # but lets the PE run the single-pass fp32 matmul mode (1 cycle/row
    # instead of 4) and keeps the BIR verifier's fp32r producer-chain check
    # happy (DMA → SBUF tile → matmul all typed fp32r).
    x_d = nc.dram_tensor("x", [NT, P, FREE], F32R, kind="ExternalInput")
    ones_d = nc.dram_tensor("ones", [P, 1], F32R, kind="ExternalInput")
    out_d = nc.dram_tensor("out", [1, KPC * M], F32, kind="ExternalOutput")

    with tile.TileContext(nc) as tc:
        with (
            tc.tile_pool(name="xin", bufs=6) as xpool,
            tc.tile_pool(name="const", bufs=1) as cpool,
            tc.tile_pool(name="res", bufs=1) as rpool,
            tc.tile_pool(name="accp", bufs=KPC, space=bass.MemorySpace.PSUM) as ppool,
        ):
            ones = cpool.tile([P, 1], F32R)
            nc.sync.dma_start(out=ones[:], in_=ones_d[:])
            res = rpool.tile([1, KPC * M], F32)

            # reps>1 wraps the body in a hardware loop — used only by the
            # benchmark harness to amplify kernel time over the ~80 ms axon
            # RPC floor; the graded kernel uses reps=1 (no loop).
            loop = tc.For_i(0, reps, 1) if reps > 1 else contextlib.nullcontext()
            with loop:
                for b in range(KPC):
                    acc = ppool.tile([1, MM_FREE], F32)
                    nmm = TILES_PER_BAG * SLICES
                    idx = 0
                    for t in range(TILES_PER_BAG):
                        xt = xpool.tile([P, FREE], F32R)
                        nc.sync.dma_start(out=xt[:], in_=x_d[b * TILES_PER_BAG + t])
                        for j in range(SLICES):
                            nc.tensor.matmul(
                                acc[0:1, :],
                                ones[:, 0:1],
                                xt[:, j * MM_FREE : (j + 1) * MM_FREE],
                                start=(idx == 0),
                                stop=(idx == nmm - 1),
                            )
                            idx += 1
                    # psum[0, c] holds sums for (n-parity c//256, m = c%256).
                    rb = res[0:1, b * M : (b + 1) * M]
                    nc.vector.tensor_copy(rb, acc[0:1, 0:M])
                    nc.vector.tensor_add(rb, rb, acc[0:1, M : 2 * M])

                nc.sync.dma_start(out=out_d[:], in_=res[:])

    nc.compile()
    return nc


def _get_nc():
    if "nc" not in _cache:
        _cache["nc"] = _build_nc()
    return _cache["nc"]


def _run_on_cores(x, trace=False):
    from concourse.bass_utils import run_bass_kernel_spmd

    nc = _get_nc()
    x = np.ascontiguousarray(np.asarray(x, dtype=np.float32))
    assert x.shape == (K, N, M)
    ones_in = np.ones((P, 1), dtype=np.float32)
    in_maps = [
        {"x": x[c * KPC : (c + 1) * KPC].reshape(NT, P, FREE), "ones": ones_in}
        for c in range(NCORES)
    ]
    r = run_bass_kernel_spmd(
        nc, in_maps, core_ids=list(range(NCORES)), trace=trace
    )
    result = np.concatenate(
        [r.results[c]["out"].reshape(KPC, M) for c in range(NCORES)], axis=0
    )
    return result, r


def kernel(x, W_left=None, b_left=None, W_right=None, b_right=None, w_last=None,
           b_last=None, **_unused):
    result, _ = _run_on_cores(x, trace=False)
    attention_map = np.ones((K, N), dtype=np.float32)
    return result, attention_map


# revision 13
# speedup vs baseline: 2.4370x; 1.0089x over previous
# Trainium2 Bass kernel for nn_AttentionModuleAggregator_48490180772189.
#
# Math: the reference applies softmax over a size-1 axis, so the attention
# map is exactly all-ones and the weighted sum collapses to
#   result[k, m] = sum_n x[k, n, m]          (x: [K=32, N=8192, M=256] f32)
#   attention_map = ones([K, N])
# The gated-MLP scores are dead code for the output, so the kernel is a pure
# memory-bound reduction over N: read 256 MiB of x once.
#
# Sharding: data-parallel over the bag axis K — 4 bags per core on 8 cores
# (32 MiB per core, contiguous slices, no host-side reduction).
#
# Per-core kernel: x shard viewed as [32, 128, 2048] (1 MiB SBUF tiles, each
# a contiguous DRAM block; 1 MiB transfers measured fastest — 256 KiB pays
# descriptor overhead, >=2 MiB loses overlap). TensorE reduces across
# partitions by matmul with a ones[128,1] stationary vector (float32r moving
# operand: 1 cycle/row vs 4 for fp32), accumulating each bag's 32 matmuls
# into one PSUM [1,512] bank. A DVE fold adds the two 256-wide halves
# (even/odd n-row parity), and one 4 KiB DMA writes the [4, 256] result.
#
# Measured on trn2 (in-NEFF loop, slope between rep counts): 96.4 us/iter
# with loads alternating across both HWDGE rings (sync+scalar), vs 100.7 us
# from one ring and 100.8 us for the bare sync-only DMA stream with no
# compute. 32 MiB/core at 358 GB/s nominal HBM is 93.7 us, so this sits
# within ~3% of the per-core HBM roofline with all 8 cores busy.

import numpy as np

K, N, M = 32, 8192, 256
NCORES = 8
KPC = K // NCORES  # bags per core
P = 128  # SBUF partitions
FREE = 2048  # free-dim per x tile (1 MiB tiles)
TILE_ELEMS = P * FREE
NT = (KPC * N * M) // TILE_ELEMS  # 16 tiles per core
TILES_PER_BAG = (N * M) // TILE_ELEMS  # 4
MM_FREE = 512  # matmul moving free dim (one PSUM bank)
SLICES = FREE // MM_FREE  # 8 matmuls per tile

_cache = {}


def _build_nc(reps=1):
    import contextlib

    import concourse.bass as bass
    import concourse.tile as tile
    from concourse import bacc, mybir

    F32 = mybir.dt.float32
    F32R = mybir.dt.float32r

    nc = bacc.Bacc("TRN2", target_bir_lowering=False, debug=False)
    # float32r end-to-end: same bytes as float32 (numpy side sees float32),
    # but lets the PE run the single-pass fp32 matmul mode (1 cycle/row
    # instead of 4) and keeps the BIR verifier's fp32r producer-chain check
    # happy (DMA → SBUF tile → matmul all typed fp32r).
    x_d = nc.dram_tensor("x", [NT, P, FREE], F32R, kind="ExternalInput")
    ones_d = nc.dram_tensor("ones", [P, 1], F32R, kind="ExternalInput")
    out_d = nc.dram_tensor("out", [1, KPC * M], F32, kind="ExternalOutput")

    with tile.TileContext(nc) as tc:
        with (
            tc.tile_pool(name="xin", bufs=6) as xpool,
            tc.tile_pool(name="const", bufs=1) as cpool,
            tc.tile_pool(name="res", bufs=1) as rpool,
            tc.tile_pool(name="accp", bufs=KPC, space=bass.MemorySpace.PSUM) as ppool,
        ):
            ones = cpool.tile([P, 1], F32R)
            nc.sync.dma_start(out=ones[:], in_=ones_d[:])
            res = rpool.tile([1, KPC * M], F32)

            # reps>1 wraps the body in a hardware loop — used only by the
            # benchmark harness to amplify kernel time over the ~80 ms axon
            # RPC floor; the graded kernel uses reps=1 (no loop).
            loop = tc.For_i(0, reps, 1) if reps > 1 else contextlib.nullcontext()
            with loop:
                for b in range(KPC):
                    acc = ppool.tile([1, MM_FREE], F32)
                    nmm = TILES_PER_BAG * SLICES
                    idx = 0
                    for t in range(TILES_PER_BAG):
                        ti = b * TILES_PER_BAG + t
                        xt = xpool.tile([P, FREE], F32R)
                        # Alternate the two HWDGE issue rings (SP + ACT):
                        # measured 4% faster than issuing all loads from sync.
                        eng = nc.sync if ti % 2 == 0 else nc.scalar
                        eng.dma_start(out=xt[:], in_=x_d[ti])
                        for j in range(SLICES):
                            nc.tensor.matmul(
                                acc[0:1, :],
                                ones[:, 0:1],
                                xt[:, j * MM_FREE : (j + 1) * MM_FREE],
                                start=(idx == 0),
                                stop=(idx == nmm - 1),
                            )
                            idx += 1
                    # psum[0, c] holds sums for (n-parity c//256, m = c%256).
                    rb = res[0:1, b * M : (b + 1) * M]
                    nc.vector.tensor_copy(rb, acc[0:1, 0:M])
                    nc.vector.tensor_add(rb, rb, acc[0:1, M : 2 * M])

                nc.sync.dma_start(out=out_d[:], in_=res[:])

    nc.compile()
    return nc


def _get_nc():
    if "nc" not in _cache:
        _cache["nc"] = _build_nc()
    return _cache["nc"]


def _run_on_cores(x, trace=False):
    from concourse.bass_utils import run_bass_kernel_spmd

    nc = _get_nc()
    x = np.ascontiguousarray(np.asarray(x, dtype=np.float32))
    assert x.shape == (K, N, M)
    ones_in = np.ones((P, 1), dtype=np.float32)
    in_maps = [
        {"x": x[c * KPC : (c + 1) * KPC].reshape(NT, P, FREE), "ones": ones_in}
        for c in range(NCORES)
    ]
    r = run_bass_kernel_spmd(
        nc, in_maps, core_ids=list(range(NCORES)), trace=trace
    )
    result = np.concatenate(
        [r.results[c]["out"].reshape(KPC, M) for c in range(NCORES)], axis=0
    )
    return result, r


def kernel(x, W_left=None, b_left=None, W_right=None, b_right=None, w_last=None,
           b_last=None, **_unused):
    result, _ = _run_on_cores(x, trace=False)
    attention_map = np.ones((K, N), dtype=np.float32)
    return result, attention_map
